# revision 3
# baseline (speedup 1.0000x reference)
"""MoE ExpertGroup kernel for Trainium2 (8 NeuronCores, expert-parallel).

Problem: E=8 experts, H=1024, I=4096, N=16384 tokens sorted by expert.
y[t] = gelu_tanh(x[t] @ w1[e(t)]) @ w2[e(t)]

Sharding: expert-parallel — core e holds expert e's weights and processes
expert e's contiguous token block (balanced routing: 2048 tokens/core).

Per-core dataflow (all matmuls in float32r — full-rate fp32 streaming):
  xT  = transpose(x)            PE transposes, 128x128 blocks
  hT  = gelu(w1.T @ xT)         MM1: lhsT=w1 tile, rhs=xT
  yT  = w2.T @ hT               MM2: lhsT=w2 tile, rhs=hT (PSUM-accumulated
                                 over 4-I-tile groups, DVE-accumulated across
                                 groups into SBUF)
  y   = transpose(yT)           PE transposes
Tokens are processed in 2 halves of 1024 to bound SBUF residency.
"""

import sys

sys.path.insert(0, "/opt/trn_rl_repo")

import numpy as np

# --- problem constants (hardcoded; kernel.py must be self-contained) ---
E = 8          # experts == cores
H = 1024       # hidden
I = 4096       # intermediate
N_TOK = 16384  # total tokens
T = N_TOK // E  # tokens per core (capacity)

P = 128
NH = 2               # token halves per core
TH = T // NH         # tokens per half (1024)
NTB = TH // 512      # 512-token blocks per half (2)
TB = 512
HB = H // P          # 8
IB = I // P          # 32
GI = 4               # I-tiles per PSUM-accumulation group
NG = IB // GI        # 8 groups
TTL = TH // P        # token tiles per half (8)

_CACHE = {}


def _build():
    import concourse.bacc as bacc
    import concourse.mybir as mybir
    import concourse.tile as tile
    from concourse.masks import make_identity

    F32 = mybir.dt.float32
    F32R = mybir.dt.float32r
    GELU = mybir.ActivationFunctionType.Gelu_apprx_tanh

    nc = bacc.Bacc("TRN2", target_bir_lowering=False, debug=False, num_devices=E)

    x = nc.dram_tensor("x", [T, H], F32, kind="ExternalInput").ap()
    w1 = nc.dram_tensor("w1", [H, I], F32R, kind="ExternalInput").ap()
    w2 = nc.dram_tensor("w2", [I, H], F32R, kind="ExternalInput").ap()
    y = nc.dram_tensor("y", [T, H], F32, kind="ExternalOutput").ap()

    with tile.TileContext(nc) as tc:
        with (
            tc.tile_pool(name="const", bufs=1) as const_pool,
            tc.tile_pool(name="xT", bufs=1) as xT_pool,
            tc.tile_pool(name="ysb", bufs=1) as y_pool,
            tc.tile_pool(name="w1p", bufs=2) as w1_pool,
            tc.tile_pool(name="w2p", bufs=2) as w2_pool,
            tc.tile_pool(name="hT", bufs=10) as hT_pool,
            tc.tile_pool(name="xstg", bufs=3) as xs_pool,
            tc.tile_pool(name="ystg", bufs=3) as ys_pool,
            tc.tile_pool(name="ph", bufs=2, space="PSUM") as ph_pool,
            tc.tile_pool(name="py", bufs=2, space="PSUM") as py_pool,
            tc.tile_pool(name="pt", bufs=4, space="PSUM") as pt_pool,
        ):
            ident = const_pool.tile([P, P], F32, tag="ident", name="ident")
            make_identity(nc, ident[:])

            for half in range(NH):
                t0 = half * TH

                # ---- phase X: load x and transpose into xT (SBUF-resident) ----
                xT = [xT_pool.tile([P, TH], F32R, tag=f"xT{k}", name=f"xT{k}") for k in range(HB)]
                for ttl in range(TTL):
                    xs = xs_pool.tile([P, H], F32, tag="xs", name="xs")
                    nc.sync.dma_start(out=xs[:], in_=x[t0 + ttl * P : t0 + (ttl + 1) * P, :])
                    for k in range(HB):
                        pt = pt_pool.tile([P, P], F32, tag="pt", name="pt")
                        nc.tensor.transpose(pt[:], xs[:, k * P : (k + 1) * P], ident[:])
                        nc.any.tensor_copy(xT[k][:, ttl * P : (ttl + 1) * P], pt[:])

                # ---- main loop ----
                ysb = [y_pool.tile([P, TH], F32, tag=f"y{h}", name=f"y{h}") for h in range(HB)]
                for g in range(NG):
                    # weight tiles for this group (w1: per k-tile; w2: per i-tile)
                    w1t = []
                    for k in range(HB):
                        wt = w1_pool.tile([P, GI * P], F32R, tag=f"w1_{k}", name=f"w1_{k}")
                        nc.sync.dma_start(
                            out=wt[:],
                            in_=w1[k * P : (k + 1) * P, g * GI * P : (g + 1) * GI * P],
                        )
                        w1t.append(wt)
                    w2t = []
                    for il in range(GI):
                        i = g * GI + il
                        wt = w2_pool.tile([P, H], F32R, tag=f"w2_{il}", name=f"w2_{il}")
                        nc.sync.dma_start(out=wt[:], in_=w2[i * P : (i + 1) * P, :])
                        w2t.append(wt)

                    for tb in range(NTB):
                        ts_ = slice(tb * TB, (tb + 1) * TB)
                        hTt = []
                        for il in range(GI):
                            ph = ph_pool.tile([P, TB], F32, tag="ph", name="ph")
                            for k in range(HB):
                                nc.tensor.matmul(
                                    ph[:],
                                    w1t[k][:, il * P : (il + 1) * P],
                                    xT[k][:, ts_],
                                    start=(k == 0),
                                    stop=(k == HB - 1),
                                )
                            ht = hT_pool.tile([P, TB], F32R, tag="ht", name="ht")
                            nc.scalar.activation(ht[:], ph[:], GELU)
                            hTt.append(ht)
                        for h in range(HB):
                            py = py_pool.tile([P, TB], F32, tag="py", name="py")
                            for il in range(GI):
                                nc.tensor.matmul(
                                    py[:],
                                    w2t[il][:, h * P : (h + 1) * P],
                                    hTt[il][:],
                                    start=(il == 0),
                                    stop=(il == GI - 1),
                                )
                            if g == 0:
                                nc.any.tensor_copy(ysb[h][:, ts_], py[:])
                            else:
                                nc.vector.tensor_add(ysb[h][:, ts_], ysb[h][:, ts_], py[:])

                # ---- phase Y: transpose yT -> y and store ----
                for ttl in range(TTL):
                    ys = ys_pool.tile([P, H], F32, tag="ys", name="ys")
                    for h in range(HB):
                        pt = pt_pool.tile([P, P], F32, tag="pt", name="pt")
                        nc.tensor.transpose(pt[:], ysb[h][:, ttl * P : (ttl + 1) * P], ident[:])
                        nc.any.tensor_copy(ys[:, h * P : (h + 1) * P], pt[:])
                    nc.sync.dma_start(out=y[t0 + ttl * P : t0 + (ttl + 1) * P, :], in_=ys[:])

    nc.compile()
    return nc


def _get_nc():
    if "nc" not in _CACHE:
        _CACHE["nc"] = _build()
    return _CACHE["nc"]


def kernel(x_sorted, w1, w2, expert_counts, local_expert_indices, **_unused):
    from concourse.bass_utils import run_bass_kernel_spmd

    x_sorted = np.ascontiguousarray(x_sorted, dtype=np.float32)
    w1 = np.ascontiguousarray(w1, dtype=np.float32)
    w2 = np.ascontiguousarray(w2, dtype=np.float32)
    counts = np.asarray(expert_counts, dtype=np.int64)

    n = x_sorted.shape[0]
    offsets = np.cumsum(counts)
    # per-token expert id, identical to reference's searchsorted
    eid = np.searchsorted(offsets, np.arange(n), side="right")

    nc = _get_nc()

    in_maps = []
    row_idx = []
    for e in range(E):
        rows = np.nonzero(eid == e)[0]
        assert len(rows) <= T, f"expert {e} overflows capacity {T}"
        xe = np.zeros((T, H), dtype=np.float32)
        xe[: len(rows)] = x_sorted[rows]
        row_idx.append(rows)
        in_maps.append({"x": xe, "w1": w1[e], "w2": w2[e]})

    res = run_bass_kernel_spmd(nc, in_maps, list(range(E))).results

    out = np.zeros((n, H), dtype=np.float32)
    for e in range(E):
        rows = row_idx[e]
        out[rows] = res[e]["y"][: len(rows)]
    return out


# revision 4
# speedup vs baseline: 1.0931x; 1.0931x over previous
"""MoE ExpertGroup kernel for Trainium2 (8 NeuronCores, expert-parallel).

Problem: E=8 experts, H=1024, I=4096, N=16384 tokens sorted by expert.
y[t] = gelu_tanh(x[t] @ w1[e(t)]) @ w2[e(t)]

Sharding: expert-parallel — core e holds expert e's weights and processes
expert e's contiguous token block (balanced routing: 2048 tokens/core).
The host ships each core's token block transposed (xT layout) and receives
the result transposed (yT) — transposition is part of the shard/unshard
step, so the device kernel is pure matmul+gelu.

Per-core dataflow (all matmuls in float32r — full-rate fp32 streaming):
  hT = gelu(w1.T @ xT)     MM1: lhsT=w1 tile [128,128], rhs=xT [128,512]
  yT = w2.T @ hT           MM2: lhsT=w2 tile, rhs=hT; PSUM-accumulated over
                            4-I-tile groups, DVE-accumulated across groups
Tokens are processed in 2 halves of 1024 to bound SBUF residency; the
second half's xT tiles are DMA'd during the first half's compute.
"""

import sys

sys.path.insert(0, "/opt/trn_rl_repo")

import numpy as np

# --- problem constants (hardcoded; kernel.py must be self-contained) ---
E = 8          # experts == cores
H = 1024       # hidden
I = 4096       # intermediate
N_TOK = 16384  # total tokens
T = N_TOK // E  # tokens per core (capacity)

P = 128
NH = 2               # token halves per core
TH = T // NH         # tokens per half (1024)
TB = 512             # token block (psum free dim)
NTB = TH // TB       # 2
HB = H // P          # 8
IB = I // P          # 32
GI = 4               # I-tiles per PSUM-accumulation group
NG = IB // GI        # 8 groups

_CACHE = {}


def _build():
    import concourse.bacc as bacc
    import concourse.mybir as mybir
    import concourse.tile as tile

    F32 = mybir.dt.float32
    F32R = mybir.dt.float32r
    GELU = mybir.ActivationFunctionType.Gelu_apprx_tanh

    nc = bacc.Bacc("TRN2", target_bir_lowering=False, debug=False, num_devices=E)

    xT = nc.dram_tensor("xT", [H, T], F32R, kind="ExternalInput").ap()
    w1 = nc.dram_tensor("w1", [H, I], F32R, kind="ExternalInput").ap()
    w2 = nc.dram_tensor("w2", [I, H], F32R, kind="ExternalInput").ap()
    yT = nc.dram_tensor("yT", [H, T], F32, kind="ExternalOutput").ap()

    with tile.TileContext(nc) as tc:
        with (
            tc.tile_pool(name="xTp", bufs=1) as xT_pool,
            tc.tile_pool(name="ysb", bufs=1) as y_pool,
            tc.tile_pool(name="w1p", bufs=2) as w1_pool,
            tc.tile_pool(name="w2p", bufs=2) as w2_pool,
            tc.tile_pool(name="hT", bufs=8) as hT_pool,
            tc.tile_pool(name="ph", bufs=4, space="PSUM") as ph_pool,
            tc.tile_pool(name="py", bufs=4, space="PSUM") as py_pool,
        ):
            for half in range(NH):
                t0 = half * TH

                # xT tiles for this half (own tags so half-2 loads overlap
                # half-1 compute)
                xTt = []
                for k in range(HB):
                    xt = xT_pool.tile([P, TH], F32R, tag=f"xT{half}_{k}", name=f"xT{half}_{k}")
                    nc.sync.dma_start(out=xt[:], in_=xT[k * P : (k + 1) * P, t0 : t0 + TH])
                    xTt.append(xt)

                ysb = [y_pool.tile([P, TH], F32, tag=f"y{h}", name=f"y{h}") for h in range(HB)]

                for g in range(NG):
                    w1t = []
                    for k in range(HB):
                        wt = w1_pool.tile([P, GI * P], F32R, tag=f"w1_{k}", name=f"w1_{k}")
                        nc.sync.dma_start(
                            out=wt[:],
                            in_=w1[k * P : (k + 1) * P, g * GI * P : (g + 1) * GI * P],
                        )
                        w1t.append(wt)
                    w2t = []
                    for il in range(GI):
                        i = g * GI + il
                        wt = w2_pool.tile([P, H], F32R, tag=f"w2_{il}", name=f"w2_{il}")
                        nc.sync.dma_start(out=wt[:], in_=w2[i * P : (i + 1) * P, :])
                        w2t.append(wt)

                    for tb in range(NTB):
                        ts_ = slice(tb * TB, (tb + 1) * TB)
                        hTt = []
                        for il in range(GI):
                            ph = ph_pool.tile([P, TB], F32, tag="ph", name="ph")
                            for k in range(HB):
                                nc.tensor.matmul(
                                    ph[:],
                                    w1t[k][:, il * P : (il + 1) * P],
                                    xTt[k][:, ts_],
                                    start=(k == 0),
                                    stop=(k == HB - 1),
                                )
                            ht = hT_pool.tile([P, TB], F32R, tag="ht", name="ht")
                            nc.scalar.activation(ht[:], ph[:], GELU)
                            hTt.append(ht)
                        for h in range(HB):
                            py = py_pool.tile([P, TB], F32, tag="py", name="py")
                            for il in range(GI):
                                nc.tensor.matmul(
                                    py[:],
                                    w2t[il][:, h * P : (h + 1) * P],
                                    hTt[il][:],
                                    start=(il == 0),
                                    stop=(il == GI - 1),
                                )
                            if g == 0:
                                nc.scalar.activation(
                                    ysb[h][:, ts_], py[:], mybir.ActivationFunctionType.Copy
                                )
                            else:
                                nc.vector.tensor_add(ysb[h][:, ts_], ysb[h][:, ts_], py[:])

                for h in range(HB):
                    nc.sync.dma_start(
                        out=yT[h * P : (h + 1) * P, t0 : t0 + TH], in_=ysb[h][:]
                    )

    nc.compile()
    return nc


def _get_nc():
    if "nc" not in _CACHE:
        _CACHE["nc"] = _build()
    return _CACHE["nc"]


def kernel(x_sorted, w1, w2, expert_counts, local_expert_indices, **_unused):
    from concourse.bass_utils import run_bass_kernel_spmd

    x_sorted = np.ascontiguousarray(x_sorted, dtype=np.float32)
    w1 = np.ascontiguousarray(w1, dtype=np.float32)
    w2 = np.ascontiguousarray(w2, dtype=np.float32)
    counts = np.asarray(expert_counts, dtype=np.int64)

    n = x_sorted.shape[0]
    offsets = np.cumsum(counts)
    # per-token expert id, identical to reference's searchsorted
    eid = np.searchsorted(offsets, np.arange(n), side="right")

    nc = _get_nc()

    in_maps = []
    row_idx = []
    for e in range(E):
        rows = np.nonzero(eid == e)[0]
        assert len(rows) <= T, f"expert {e} overflows capacity {T}"
        xe = np.zeros((T, H), dtype=np.float32)
        xe[: len(rows)] = x_sorted[rows]
        row_idx.append(rows)
        in_maps.append(
            {"xT": np.ascontiguousarray(xe.T), "w1": w1[e], "w2": w2[e]}
        )

    res = run_bass_kernel_spmd(nc, in_maps, list(range(E))).results

    out = np.zeros((n, H), dtype=np.float32)
    for e in range(E):
        rows = row_idx[e]
        ye = np.ascontiguousarray(res[e]["yT"].T)
        out[rows] = ye[: len(rows)]
    return out
